# revision 18
# baseline (speedup 1.0000x reference)
"""Trainium2 Bass kernel for additive-attention pooling.

  reference math (per sample b):
      scores = tanh(X[b] @ W) @ u          # (T,)
      att    = softmax(scores)             # (T,)
      out[b] = att @ X[b]                  # (D,)

  B, T, D, CTX = 32, 8192, 256, 128.

Strategy: data-parallel over batch, 4 samples per core on 8 cores.
X is uploaded twice in fp16 (same total bytes as fp32 once):
  XT - transposed  [p=d%128, h=d//128, t]  -> feeds the d-contraction (scores)
  XN - natural     [p=t%128, j=t//128, d]  -> feeds the t-contraction (pooling)
All heavy compute runs on the tensor engine; tanh/exp on the scalar engine.
Scores for chunk i are routed to PSUM partition row i by using a masked copy
of u (only output-column i nonzero) and accumulating all 16 chunk matmuls
into one [16, 512] PSUM tile.  exp and its row-sums are one fused ACT op.
softmax normalization (divide by the scalar sum) is applied to the final
256-vector instead of the 8192 attention weights.
"""

import numpy as np

B, T, D, CTX = 32, 8192, 256, 128
N_CORES = 8
SPB = B // N_CORES  # samples per core

# Set True (e.g. from test.py) to capture an NTFF profile; LAST_RESULTS then
# holds the BassKernelResults with exec_time_ns.
TRACE = False
LAST_RESULTS = None
_NPIECE_XT = 8   # DMA pieces for XT per sample (512 KB each)
_NPIECE_XN = 4   # DMA pieces for XN per sample (1 MB each)

_prog_cache = {}


def _build_program(spb, t_len, nch, w16, umask, ident, ones_col, repeat=1,
                   parts="all", hw_loop=0):
    """Build + compile the SPMD Bass program.

    spb: samples per core; t_len: time length; nch: score chunks (chunk = t_len/nch).
    w16   [2, 128, CTX]   fp16  W split into d-halves, lhsT layout [d, c]
    umask [nch, CTX, nch] fp16  umask[i, c, m] = u[c] * (m == i)
    ident [nch, nch]      fp16  identity for PE transposes
    ones_col [nch, 1]     fp32  ones, for the total-sum matmul
    """
    import concourse.bass as bass
    import concourse.tile as tile
    from concourse import bacc, mybir

    f16 = mybir.dt.float16
    f32 = mybir.dt.float32
    AF = mybir.ActivationFunctionType

    ch = t_len // nch          # elements per score chunk (512)
    nt = t_len // 128          # number of 128-row t-tiles (64)
    jpc = ch // 128            # t-tiles per score chunk (4)
    np_xt = _NPIECE_XT if t_len == 8192 else max(1, t_len // 4096)
    np_xn = _NPIECE_XN if t_len == 8192 else max(1, t_len // 4096)
    tpp = t_len // np_xt       # t per xt piece
    jpp = nt // np_xn          # j per xn piece

    nc = bacc.Bacc("TRN2", target_bir_lowering=False, debug=False,
                   num_devices=N_CORES)

    XT = nc.dram_tensor("XT", [spb, 128, 2, t_len], f16, kind="ExternalInput")
    XN = nc.dram_tensor("XN", [spb, 128, nt, D], f16, kind="ExternalInput")
    OUT = nc.dram_tensor("OUT", [spb, D], f32, kind="ExternalOutput")

    W_h = nc.inline_tensor(w16, name="Wc")
    UM_h = nc.inline_tensor(umask, name="UMc")
    ID_h = nc.inline_tensor(ident, name="IDc")
    ONE_h = nc.inline_tensor(ones_col, name="ONEc")

    with tile.TileContext(nc) as tc:
        with (
            tc.tile_pool(name="const", bufs=1) as cpool,
            tc.tile_pool(name="xt", bufs=2 * np_xt) as xt_pool,
            tc.tile_pool(name="xn", bufs=2 * np_xn) as xn_pool,
            tc.tile_pool(name="y", bufs=3) as y_pool,
            tc.tile_pool(name="sm", bufs=2) as sm_pool,
            tc.tile_pool(name="res", bufs=1) as res_pool,
            tc.tile_pool(name="py", bufs=2, space="PSUM") as py_pool,
            tc.tile_pool(name="ps", bufs=2, space="PSUM") as ps_pool,
            tc.tile_pool(name="pt", bufs=1, space="PSUM") as pt_pool,
            tc.tile_pool(name="po", bufs=2, space="PSUM") as po_pool,
        ):
            # ---- constants (resident) ----
            w_sb = cpool.tile([128, 2, CTX], f16, tag="w")
            nc.sync.dma_start(w_sb[:], W_h.ap().rearrange("h p c -> p h c"))
            um_sb = cpool.tile([CTX, nch, nch], f16, tag="um")
            nc.sync.dma_start(um_sb[:], UM_h.ap().rearrange("i c m -> c i m"))
            id_sb = cpool.tile([nch, nch], f16, tag="id")
            nc.sync.dma_start(id_sb[:], ID_h.ap())
            one_sb = cpool.tile([nch, 1], f32, tag="one")
            nc.sync.dma_start(one_sb[:], ONE_h.ap())

            def _one_repeat():
              out_sb = res_pool.tile([1, spb * D], f32, tag="out")
              for s in range(spb):
                # ---- load this sample's X in both layouts (2 MB pieces) ----
                xt_sbs = []
                xn_sbs = []
                for k in range(np_xt):
                    xt_k = xt_pool.tile([128, 2, tpp], f16, tag="xt")
                    if parts in ("all", "dma"):
                        nc.sync.dma_start(
                            xt_k[:], XT.ap()[s][:, :, k * tpp:(k + 1) * tpp])
                    else:
                        nc.vector.memset(xt_k[:, 0, 0:1], 0)
                    xt_sbs.append(xt_k)
                for k in range(np_xn):
                    xn_k = xn_pool.tile([128, jpp, D], f16, tag="xn")
                    if parts in ("all", "dma"):
                        nc.sync.dma_start(
                            xn_k[:], XN.ap()[s][:, k * jpp:(k + 1) * jpp, :])
                    else:
                        nc.vector.memset(xn_k[:, 0, 0:1], 0)
                    xn_sbs.append(xn_k)
                if parts == "dma":
                    if s == 0:
                        nc.vector.memset(out_sb[:, 0:1], 0)
                    continue

                # ---- pass A: scores, chunk by chunk ----
                # ps accumulates all chunks; chunk i lands in partition row i
                ps = ps_pool.tile([nch, ch], f32, tag="ps")
                for i in range(nch):
                    t0 = i * ch
                    kp = t0 // tpp            # which xt piece
                    o0 = t0 - kp * tpp
                    py = py_pool.tile([CTX, ch], f32, tag="py")
                    nc.tensor.matmul(py[:], w_sb[:, 0], xt_sbs[kp][:, 0, o0:o0 + ch],
                                     start=True, stop=False)
                    nc.tensor.matmul(py[:], w_sb[:, 1], xt_sbs[kp][:, 1, o0:o0 + ch],
                                     start=False, stop=True)
                    y_sb = y_pool.tile([CTX, ch], f16, tag="y")
                    nc.scalar.activation(y_sb[:], py[:], AF.Tanh)
                    nc.tensor.matmul(ps[:], um_sb[:, i, :], y_sb[:],
                                     start=(i == 0), stop=(i == nch - 1),
                                     skip_group_check=True)

                # ---- softmax pieces ----
                ex_sb = sm_pool.tile([nch, ch], f16, tag="ex")
                sums = sm_pool.tile([nch, 1], f32, tag="sums")
                nc.scalar.activation(ex_sb[:], ps[:], AF.Exp, accum_out=sums[:])

                pt = pt_pool.tile([128, jpc, nch], f16, tag="pt")
                for q in range(jpc):
                    nc.tensor.transpose(pt[:, q, :], ex_sb[:, q * 128:(q + 1) * 128],
                                        id_sb[:])
                att = sm_pool.tile([128, jpc * nch], f16, tag="att")
                nc.vector.tensor_copy(att[:], pt[:])

                ptot = pt_pool.tile([1, 1], f32, tag="ptot")
                nc.tensor.matmul(ptot[:], one_sb[:], sums[:], start=True, stop=True)
                inv = sm_pool.tile([1, 1], f32, tag="inv")
                nc.vector.reciprocal(inv[:], ptot[:])

                # ---- pass B: weighted sum over time ----
                po = po_pool.tile([1, D], f32, tag="po")
                for j in range(nt):
                    b_, q_ = divmod(j, jpc)
                    col = q_ * nch + b_
                    kp = j // jpp
                    nc.tensor.matmul(po[:], att[:, col:col + 1],
                                     xn_sbs[kp][:, j - kp * jpp, :],
                                     start=(j == 0), stop=(j == nt - 1))

                nc.vector.tensor_scalar(out_sb[0:1, s * D:(s + 1) * D], po[:],
                                        inv[:], None, mybir.AluOpType.mult)

              nc.sync.dma_start(OUT.ap().rearrange("s d -> () (s d)"), out_sb[:])

            if hw_loop:
                with tc.For_i(0, hw_loop, 1,
                              hint_engines=(mybir.EngineType.PE,
                                            mybir.EngineType.Activation,
                                            mybir.EngineType.SP,
                                            mybir.EngineType.DVE)):
                    _one_repeat()
            else:
                for _r in range(repeat):
                    _one_repeat()

    nc.compile()
    return nc


def _prep_consts(W, u, nch):
    w16 = np.ascontiguousarray(
        W.astype(np.float16).reshape(2, 128, CTX))
    u16 = u.astype(np.float16).reshape(CTX)
    umask = np.zeros((nch, CTX, nch), dtype=np.float16)
    for i in range(nch):
        umask[i, :, i] = u16
    ident = np.eye(nch, dtype=np.float16)
    ones_col = np.ones((nch, 1), dtype=np.float32)
    return w16, umask, ident, ones_col


def _pack_inputs(X16):
    """X16: [nb, T, D] fp16 -> (XT [nb,128,2,T], XN [nb,128,T//128,D])."""
    nb, t_len, d = X16.shape
    # XT[s, p, h, t] = X[s, t, h*128+p]
    xt = np.ascontiguousarray(
        X16.transpose(0, 2, 1).reshape(nb, 2, 128, t_len).transpose(0, 2, 1, 3))
    # XN[s, p, j, d] = X[s, j*128+p, d]
    xn = np.ascontiguousarray(
        X16.reshape(nb, t_len // 128, 128, d).transpose(0, 2, 1, 3))
    return xt, xn


def kernel(X, W, u):
    global LAST_RESULTS
    from concourse.bass_utils import run_bass_kernel_spmd

    X = np.asarray(X)
    W = np.asarray(W)
    u = np.asarray(u)
    assert X.shape == (B, T, D) and W.shape == (D, CTX) and u.shape == (CTX, 1), (
        X.shape, W.shape, u.shape)

    nch = 16
    key = (SPB, T, nch, W.tobytes(), u.tobytes())
    if key not in _prog_cache:
        _prog_cache.clear()
        _prog_cache[key] = _build_program(
            SPB, T, nch, *_prep_consts(W, u, nch))
    nc = _prog_cache[key]

    X16 = np.asarray(X).astype(np.float16)
    in_maps = []
    for c in range(N_CORES):
        xt, xn = _pack_inputs(X16[c * SPB:(c + 1) * SPB])
        in_maps.append({"XT": xt, "XN": xn})

    try:
        res = run_bass_kernel_spmd(nc, in_maps, core_ids=list(range(N_CORES)),
                                   trace=TRACE)
    except (ImportError, ModuleNotFoundError):
        # NTFF profiling hook unavailable in this axon build; run untraced.
        res = run_bass_kernel_spmd(nc, in_maps, core_ids=list(range(N_CORES)),
                                   trace=False)
    LAST_RESULTS = res
    return np.concatenate([r["OUT"] for r in res.results], axis=0)
